# revision 1
# baseline (speedup 1.0000x reference)
"""BitSelfAttention on 8 TRN2 NeuronCores.

Sharding: core c handles batch b = c//2 and head-group hg = c%2 (8 of 16 heads).
Each core computes its 8 heads' QKV projections + causal attention + its slice
of the o_proj GEMM, producing a partial output (transposed, [D, T], fp32).
Host: pre-quantizes BitLinear weights (ternary * gamma, exact in bf16),
pre-transposes operands into matmul-friendly layouts, and sums the two
head-group partials per batch at the end.

Device layouts (per core):
  xT   [D, T]  bf16 : x[b].T              (rhs for Q/K/V^T projections)
  wqT  [D, F]  bf16 : w_q_eff[hg-rows].T  (stationary tiles for Q^T proj)
  wkT  [D, F]  bf16
  wvT  [D, F]  bf16
  woT  [F, D]  bf16 : w_o_eff[:, hg-cols].T (stationary tiles for o_proj)
  cmask[4, 128, 512] bf16 : causal masks for the 4 diagonal offsets
  outT [D, T]  fp32 : partial output, transposed

Per head h: Q^T,K^T [dh=128, T] (dh-major), V^T transposed on the PE into
token-major V tiles. Attention computed as S^T = K^T_tile.T @ Q^T_block so
softmax rows land on the free axis; P^T = exp(S^T*scale) (ACT, PSUM->SBUF
bf16); key-tile partial row-sums accumulate in fp32 on the vector engine and
one all-ones stationary matmul per block reduces across partitions while
broadcasting the result to every partition (so normalization needs no
cross-partition broadcast); O^T = V_tile.T @ P^T accumulated over key tiles;
normalize with fast-reciprocal+multiply during PSUM eviction. o_proj consumes
O^T tiles directly as stationary operands, producing outT; its per-token-block
chains double as PE fill work zipped into the last head's attention, just as
each head's projection chains are zipped into the previous head's attention
(the attention inner loop is otherwise exp-latency-gated on the in-order PE).
"""

import math

import ml_dtypes
import numpy as np

import concourse.mybir as mybir
import concourse.tile as tile
from concourse import bacc
from concourse import bass_utils
from concourse.masks import make_identity

BF16 = mybir.dt.bfloat16
F32 = mybir.dt.float32

D_MODEL = 2048
N_HEAD = 16
D_HEAD = 128
B = 4
T_FULL = 2048
N_CORES = 8
F_LOC = D_MODEL // 2  # features per core (8 heads)


def build_bass(T=T_FULL, D=D_MODEL, F=F_LOC, debug=False):
    """Build the single-core program (SPMD across 8 cores via input data)."""
    P = 128
    KD = D // P      # contraction 128-tiles
    TT = T // P      # token 128-tiles
    TB = T // 512    # token 512-blocks
    H = F // P       # local heads
    MT = D // P      # output-dmodel 128-tiles
    KT_PER_B = 512 // P
    SCALE = 1.0 / math.sqrt(D_HEAD)

    nc = bacc.Bacc("TRN2", target_bir_lowering=False, debug=debug,
                   num_devices=N_CORES)
    xT_d = nc.dram_tensor("xT", [D, T], BF16, kind="ExternalInput").ap()
    # weights pre-tiled on host into the exact SBUF layouts (contiguous DMAs):
    #   wqT/wkT/wvT: [H, 128, KD*128] with [h, p, kd*128+f] = w_eff[h*128+f, kd*128+p]
    #   woT:         [MT, 128, H*128] with [m, p, h*128+j] = wo_eff[m*128+j, h*128+p]
    H_ = F // P
    MT_ = D // P
    KD_ = D // P
    wqT_d = nc.dram_tensor("wqT", [H_, P, KD_ * P], BF16,
                           kind="ExternalInput").ap()
    wkT_d = nc.dram_tensor("wkT", [H_, P, KD_ * P], BF16,
                           kind="ExternalInput").ap()
    wvT_d = nc.dram_tensor("wvT", [H_, P, KD_ * P], BF16,
                           kind="ExternalInput").ap()
    woT_d = nc.dram_tensor("woT", [MT_, P, H_ * P], BF16,
                           kind="ExternalInput").ap()
    cm_d = nc.dram_tensor("cmask", [4, P, 512], BF16, kind="ExternalInput").ap()
    out_d = nc.dram_tensor("outT", [D, T], F32, kind="ExternalOutput").ap()

    with tile.TileContext(nc) as tc:
        with (
            tc.tile_pool(name="big", bufs=1) as big,
            tc.tile_pool(name="work", bufs=2) as work,
            tc.tile_pool(name="psS", bufs=3, space="PSUM") as psS,
            tc.tile_pool(name="psO", bufs=2, space="PSUM") as psO,
            tc.tile_pool(name="psR", bufs=1, space="PSUM") as psR,
            tc.tile_pool(name="psP", bufs=2, space="PSUM") as psP,
        ):
            # ---- persistent inputs (head-0 weights first: first MMs need them)
            wvh0 = work.tile([P, KD, P], BF16, name="wvh0", tag="wvh")
            nc.sync.dma_start(out=wvh0.rearrange("p kd f -> p (kd f)"),
                              in_=wvT_d[0])
            ones = big.tile([P, P], BF16, name="ones_sb", tag="ones", bufs=1)
            nc.vector.memset(ones, 1.0)
            ident = big.tile([P, P], BF16, name="ident_sb", tag="ident", bufs=1)
            make_identity(nc, ident)
            xt = []
            for kd in range(KD):
                xti = big.tile([P, T], BF16, name=f"xt{kd}", tag="xt", bufs=KD)
                nc.sync.dma_start(out=xti, in_=xT_d[kd * P:(kd + 1) * P, :])
                xt.append(xti)
            cmask = big.tile([P, 4, 512], BF16, name="cmask_sb", tag="cmask",
                             bufs=1)
            for i in range(4):
                nc.sync.dma_start(out=cmask[:, i, :], in_=cm_d[i])
            ot = [big.tile([P, T], BF16, name=f"ot{h}", tag="ot", bufs=H)
                  for h in range(H)]

            # ---- per-head pipeline with cross-head fill interleaving.
            # The attention inner loop is ACT(exp)-gated by ~40ns/iter; we
            # pump one projection matmul of the NEXT head between attention
            # iterations so the (in-order) PE always has fill work.
            def load_head_weights(h, wvh=None):
                if wvh is None:
                    wvh = work.tile([P, KD, P], BF16, name=f"wvh{h}",
                                    tag="wvh")
                    nc.sync.dma_start(out=wvh.rearrange("p kd f -> p (kd f)"),
                                      in_=wvT_d[h])
                wqh = work.tile([P, KD, P], BF16, name=f"wqh{h}", tag="wqh")
                nc.sync.dma_start(out=wqh.rearrange("p kd f -> p (kd f)"),
                                  in_=wqT_d[h])
                wkh = work.tile([P, KD, P], BF16, name=f"wkh{h}", tag="wkh")
                nc.sync.dma_start(out=wkh.rearrange("p kd f -> p (kd f)"),
                                  in_=wkT_d[h])
                return wqh, wkh, wvh

            def load_qk_weights(h):
                wqh = work.tile([P, KD, P], BF16, name=f"wqh{h}", tag="wqh")
                nc.sync.dma_start(out=wqh.rearrange("p kd f -> p (kd f)"),
                                  in_=wqT_d[h])
                wkh = work.tile([P, KD, P], BF16, name=f"wkh{h}", tag="wkh")
                nc.sync.dma_start(out=wkh.rearrange("p kd f -> p (kd f)"),
                                  in_=wkT_d[h])
                return wqh, wkh

            def alloc_head_tiles(h):
                vT = work.tile([P, T], BF16, name=f"vT{h}", tag="vT")
                vh = work.tile([P, TT, P], BF16, name=f"vh{h}", tag="vh")
                qt_ = work.tile([P, T], BF16, name=f"qt{h}", tag="qt")
                kt_ = work.tile([P, T], BF16, name=f"kt{h}", tag="kt")
                return vT, vh, qt_, kt_

            def proj_fill_gen(ws, tiles):
                """V^T then Q^T then K^T projection chains, yielding after
                every matmul so the caller can interleave them."""
                wqh, wkh, wvh = ws
                vT, vh, qt_, kt_ = tiles
                for wh, dst in ((wvh, vT), (wqh, qt_), (wkh, kt_)):
                    for tb in range(TB):
                        ts_ = slice(tb * 512, (tb + 1) * 512)
                        ps = psP.tile([P, 512], F32, name="psfill", tag="psp")
                        for kd in range(KD):
                            nc.tensor.matmul(ps, lhsT=wh[:, kd, :],
                                             rhs=xt[kd][:, ts_],
                                             start=(kd == 0),
                                             stop=(kd == KD - 1))
                            yield
                        nc.vector.tensor_copy(out=dst[:, ts_], in_=ps)

            def pump(gen, n):
                for _ in range(n):
                    try:
                        next(gen)
                    except StopIteration:
                        return False
                return True

            def pump_n(gen, n):
                c = 0
                for _ in range(n):
                    try:
                        next(gen)
                        c += 1
                    except StopIteration:
                        break
                return c

            def oproj_nb_gen(nb):
                """o_proj chains for one token block (needs all heads' ot
                columns of that block only), yielding per matmul."""
                ns = slice(nb * 512, (nb + 1) * 512)
                for m in range(MT):
                    woh = work.tile([P, H, P], BF16, name=f"woh{nb}_{m}",
                                    tag="woh", bufs=4)
                    nc.sync.dma_start(out=woh.rearrange("p h f -> p (h f)"),
                                      in_=woT_d[m])
                    yield  # let attention matmuls cover the woh DMA latency
                    ps = psP.tile([P, 512], F32, name="psout", tag="psp")
                    for hh in range(H):
                        nc.tensor.matmul(ps, lhsT=woh[:, hh, :],
                                         rhs=ot[hh][:, ns],
                                         start=(hh == 0), stop=(hh == H - 1))
                        yield
                    stg = work.tile([P, 512], F32, name="ostage", tag="ostage",
                                    bufs=4)
                    nc.vector.tensor_copy(out=stg, in_=ps)
                    nc.sync.dma_start(out=out_d[m * P:(m + 1) * P, ns],
                                      in_=stg)

            # head-0 Q/K weights and head-1 weights load after xt (the V^T
            # chains consume xt first; the Q chains run ~4 chain-times later)
            ws_list = [None] * (H + 2)
            wqh0, wkh0 = load_qk_weights(0)
            ws_list[0] = (wqh0, wkh0, wvh0)
            if H > 1:
                ws_list[1] = load_head_weights(1)
            cur_tiles = alloc_head_tiles(0)
            g0 = proj_fill_gen(ws_list[0], cur_tiles)
            while pump(g0, 1):
                pass

            fills = []

            def pump_fills(n):
                while n > 0 and fills:
                    n -= pump_n(fills[0], n)
                    if n > 0:
                        fills.pop(0)

            for h in range(H):
                vT, vh, qt_, kt_ = cur_tiles
                # prefetch weights two heads ahead so fill matmuls never
                # wait on their DMA (a blocked fill stalls the in-order PE)
                if h + 2 < H:
                    ws_list[h + 2] = load_head_weights(h + 2)
                if h + 1 < H:
                    next_tiles = alloc_head_tiles(h + 1)
                    fills.append(proj_fill_gen(ws_list[h + 1], next_tiles))
                else:
                    next_tiles = None

                def emit_transpose(kt):
                    # lives in the psS pool: psP slots are held long by
                    # in-flight interleaved fill chains
                    pst = psS.tile([P, 512], BF16, name="pst", tag="pss")
                    nc.tensor.transpose(pst[:, 0:P],
                                        vT[:, kt * P:(kt + 1) * P], ident)
                    nc.vector.tensor_copy(out=vh[:, kt, :], in_=pst[:, 0:P])

                # causal attention, S^T layout (keys on partitions).
                # Diagonal tiles (kt = 4*qb+di) only contribute to query
                # columns >= 128*di of the block; narrow S/exp/O/R to the
                # live columns. Only the first 128 columns of a (narrowed)
                # diagonal tile are triangular; the rest are fully allowed.
                for qb in range(TB):
                    nkt = KT_PER_B * (qb + 1)
                    for kt in range(KT_PER_B * qb, nkt):
                        emit_transpose(kt)
                    psO_t = psO.tile([P, 512], F32, name="psodt", tag="pso")
                    racc = work.tile([P, 512], F32, name="racc", tag="racc")
                    for kt in range(nkt):
                        di = kt - KT_PER_B * qb
                        c0 = max(di, 0) * P  # first live query column
                        w = 512 - c0
                        qs = slice(qb * 512 + c0, (qb + 1) * 512)
                        psS_t = psS.tile([P, 512], F32, name="pssc", tag="pss")
                        nc.tensor.matmul(psS_t[:, :w],
                                         lhsT=kt_[:, kt * P:(kt + 1) * P],
                                         rhs=qt_[:, qs],
                                         start=True, stop=True)
                        pt = work.tile([P, 512], BF16, name="pexp", tag="pt",
                                       bufs=6)
                        nc.scalar.activation(
                            out=pt[:, :w], in_=psS_t[:, :w],
                            func=mybir.ActivationFunctionType.Exp, scale=SCALE)
                        if di >= 0:
                            nc.vector.tensor_mul(pt[:, :P], pt[:, :P],
                                                 cmask[:, 0, :P])
                        nc.tensor.matmul(psO_t[:, c0:], lhsT=vh[:, kt, :],
                                         rhs=pt[:, :w],
                                         start=(kt == 0), stop=(kt == nkt - 1),
                                         skip_group_check=True)
                        # fp32 running key-tile sum on DVE (hidden behind the
                        # exp pacing); one ones-matmul at the end reduces
                        # across partitions and broadcasts
                        if kt == 0:
                            nc.vector.tensor_copy(out=racc, in_=pt)
                        else:
                            nc.vector.tensor_add(racc[:, c0:], racc[:, c0:],
                                                 pt[:, :w])
                        pump_fills(1 + (kt & 1))
                    raccb = work.tile([P, 512], BF16, name="raccb", tag="raccb")
                    nc.vector.tensor_copy(out=raccb, in_=racc)
                    psR_t = psR.tile([P, 512], F32, name="psrow", tag="psr")
                    nc.tensor.matmul(psR_t, lhsT=ones, rhs=raccb,
                                     start=True, stop=True)
                    rec = work.tile([P, 512], F32, name="rec", tag="rec")
                    nc.vector.reciprocal_approx_fast(out=rec, in_=psR_t)
                    nc.vector.tensor_mul(ot[h][:, qb * 512:(qb + 1) * 512],
                                         psO_t, rec)
                    if h == H - 1:
                        # this token block's ot columns are now complete for
                        # every head: its o_proj chains become fill work
                        fills.append(oproj_nb_gen(qb))
                    pump_fills(4)
                if h < H - 1:
                    # finish next head's projections before its attention
                    while fills:
                        pump_fills(64)
                cur_tiles = next_tiles
            # drain remaining o_proj work
            while fills:
                pump_fills(64)

    nc.compile()
    return nc


def _bitlinear_eff(w):
    """Forward-effective BitLinear weight: clip(round(w/gamma),-1,1)*gamma."""
    w = np.asarray(w, dtype=np.float32)
    gamma = max(np.float32(np.abs(w).mean()), np.float32(1e-5))
    q = np.clip(np.round(w / gamma), -1.0, 1.0).astype(np.float32)
    return q * gamma


def _causal_masks():
    k = np.arange(128)[:, None]
    q = np.arange(512)[None, :]
    m = np.stack([(k <= q - 128 * i) for i in range(4)]).astype(np.float32)
    return m.astype(ml_dtypes.bfloat16)


def _tile_qkv(w_shard):
    """[F, D] -> [H, 128, KD*128]: [h, p, kd*128+f] = w_shard[h*128+f, kd*128+p]."""
    Fs, Ds = w_shard.shape
    a = w_shard.reshape(Fs // 128, 128, Ds // 128, 128)  # [h, f, kd, p]
    a = a.transpose(0, 3, 2, 1).reshape(Fs // 128, 128, Ds)
    return np.ascontiguousarray(a)


def _tile_wo(wo_shard):
    """[D, F] -> [MT, 128, H*128]: [m, p, h*128+j] = wo_shard[m*128+j, h*128+p]."""
    Ds, Fs = wo_shard.shape
    a = wo_shard.reshape(Ds // 128, 128, Fs // 128, 128)  # [m, j, h, p]
    a = a.transpose(0, 3, 2, 1).reshape(Ds // 128, 128, Fs)
    return np.ascontiguousarray(a)


def _prep_inputs(x, wq, wk, wv, wo):
    bf = ml_dtypes.bfloat16
    x = np.asarray(x, dtype=np.float32)
    effs = {n: _bitlinear_eff(w) for n, w in
            (("wq", wq), ("wk", wk), ("wv", wv), ("wo", wo))}
    cmask = _causal_masks()
    xTs = [np.ascontiguousarray(x[b].T).astype(bf) for b in range(B)]
    shards = []
    for hg in range(2):
        rows = slice(hg * F_LOC, (hg + 1) * F_LOC)
        shards.append({
            "wqT": _tile_qkv(effs["wq"][rows, :]).astype(bf),
            "wkT": _tile_qkv(effs["wk"][rows, :]).astype(bf),
            "wvT": _tile_qkv(effs["wv"][rows, :]).astype(bf),
            "woT": _tile_wo(effs["wo"][:, rows]).astype(bf),
        })
    in_maps = []
    for c in range(N_CORES):
        b, hg = c // 2, c % 2
        m = {"xT": xTs[b], "cmask": cmask}
        m.update(shards[hg])
        in_maps.append(m)
    return in_maps


_NC_CACHE = {}


def _get_nc():
    if "nc" not in _NC_CACHE:
        _NC_CACHE["nc"] = build_bass()
    return _NC_CACHE["nc"]


def run(x, wq, wk, wv, wo, trace=False):
    nc = _get_nc()
    in_maps = _prep_inputs(x, wq, wk, wv, wo)
    res = bass_utils.run_bass_kernel_spmd(
        nc, in_maps, core_ids=list(range(N_CORES)), trace=trace)
    out = np.empty((B, T_FULL, D_MODEL), dtype=np.float32)
    for b in range(B):
        out[b] = (res.results[2 * b]["outT"]
                  + res.results[2 * b + 1]["outT"]).T
    return out, res


def kernel(x, wq, wk, wv, wo):
    out, _ = run(x, wq, wk, wv, wo)
    return out



# revision 2
# speedup vs baseline: 1.1078x; 1.1078x over previous
"""BitSelfAttention on 8 TRN2 NeuronCores — fp8 DoubleRow hybrid.

Sharding: core c handles batch b = c//2 and head-group hg = c%2 (8 of 16
heads). Each core computes its 8 heads' QKV projections + causal attention +
its slice of the o_proj GEMM, producing a partial output ([D, T], fp32);
host sums the two head-group partials per batch and applies the folded
BitLinear gammas.

The device works in pure-ternary units: BitLinear weights are sent as their
ternary {-1,0,+1} values (EXACT in fp8e4), gamma_q*gamma_k is folded into the
softmax exp scale, and 2*gamma_v*gamma_o is applied on the host (the V
weights carry an extra 0.5 — exact in bf16 — so that |v~| stays < 240, the
TRN fp8e4 saturation point).

fp8 DoubleRow (2 contraction subtiles per MM, ~1.8x MAC throughput) is used
where a numpy error simulation showed it is safe (sim rel err 0.92% vs the
2e-2 gate; all-fp8 variants fail):
  - Q/K projections:  fp8 DR (x as e4m3 moving operand, ternary w stationary)
  - V projection:     bf16, x STATIONARY per token-tile so V lands
                      token-major directly — kills the 128 PE transposes the
                      baseline spent ~35us on. w_v is the moving operand.
  - QK^T scores:      bf16 (contraction is d_head=128 — DR inapplicable)
  - P@V off-diagonal: fp8 DR (exp emits e4m3, V kept in an fp8 copy)
  - P@V diagonal:     bf16 (peaked early-token rows need accurate V; this is
                      what keeps token<512 rows at bf16 accuracy)
  - o_proj block 0:   bf16 (peaked rows), blocks 1-3: fp8 DR
Row-sums accumulate in fp32 on DVE; one all-ones stationary matmul per block
reduces across partitions and broadcasts; normalization via fast reciprocal
multiply at PSUM eviction. Fill interleaving (projection/V/o_proj chains
pumped between exp-gated attention ops) mirrors the proven baseline.
"""

import math

import ml_dtypes
import numpy as np

import concourse.mybir as mybir
import concourse.tile as tile
from concourse import bacc
from concourse import bass_utils

BF16 = mybir.dt.bfloat16
F32 = mybir.dt.float32
F8 = mybir.dt.float8e4
DR = mybir.MatmulPerfMode.DoubleRow
EXP = mybir.ActivationFunctionType.Exp

D_MODEL = 2048
N_HEAD = 16
D_HEAD = 128
B = 4
T_FULL = 2048
N_CORES = 8
F_LOC = D_MODEL // 2  # features per core (8 heads)


def build_bass(scale, T=T_FULL, D=D_MODEL, F=F_LOC, debug=False):
    """Build the single-core program (SPMD across 8 cores via input data)."""
    P = 128
    KD = D // P      # contraction 128-tiles (16)
    JD = KD // 2     # DR pairs over contraction (8)
    TT = T // P      # token 128-tiles
    TB = T // 512    # token 512-blocks
    H = F // P       # local heads (8)
    MT = D // P      # output-dmodel 128-tiles (16)
    KT_PER_B = 512 // P

    nc = bacc.Bacc("TRN2", target_bir_lowering=False, debug=debug,
                   num_devices=N_CORES)
    x8_d = nc.dram_tensor("x8", [D, T], F8, kind="ExternalInput").ap()
    xtc_d = nc.dram_tensor("xtc", [TT, P, KD * P], BF16,
                           kind="ExternalInput").ap()
    # wq8/wk8: [H, 128, KD*128] ternary fp8, [h, p, kd*128+f] = t[h*128+f, kd*128+p]
    wq8_d = nc.dram_tensor("wq8", [H, P, KD * P], F8, kind="ExternalInput").ap()
    wk8_d = nc.dram_tensor("wk8", [H, P, KD * P], F8, kind="ExternalInput").ap()
    # wvm: [KD, 128, F] bf16 = 0.5 * tv[f, kd*128+p]  (moving operand)
    wvm_d = nc.dram_tensor("wvm", [KD, P, F], BF16, kind="ExternalInput").ap()
    # wo: [MT, 128, H*128], [m, p, h*128+j] = to[m*128+j, h*128+p]
    wo8_d = nc.dram_tensor("wo8", [MT, P, H * P], F8, kind="ExternalInput").ap()
    wob_d = nc.dram_tensor("wob", [MT, P, H * P], BF16,
                           kind="ExternalInput").ap()
    cm_d = nc.dram_tensor("cmask", [P, P], BF16, kind="ExternalInput").ap()
    out_d = nc.dram_tensor("outT", [D, T], F32, kind="ExternalOutput").ap()

    with tile.TileContext(nc) as tc:
        with (
            tc.tile_pool(name="big", bufs=1) as big,
            tc.tile_pool(name="work", bufs=2) as work,
            tc.tile_pool(name="psS", bufs=3, space="PSUM") as psS,
            tc.tile_pool(name="psO", bufs=2, space="PSUM") as psO,
            tc.tile_pool(name="psR", bufs=1, space="PSUM") as psR,
            tc.tile_pool(name="psP", bufs=2, space="PSUM") as psP,
        ):
            # ---- persistent inputs (head-0 weights first: first MMs need them)
            def load_qk_weights(h):
                wq_t = work.tile([P, KD, P], F8, name=f"wq{h}", tag="wq8")
                nc.sync.dma_start(out=wq_t.rearrange("p kd f -> p (kd f)"),
                                  in_=wq8_d[h])
                wk_t = work.tile([P, KD, P], F8, name=f"wk{h}", tag="wk8")
                nc.sync.dma_start(out=wk_t.rearrange("p kd f -> p (kd f)"),
                                  in_=wk8_d[h])
                return wq_t, wk_t

            ws_list = [None] * (H + 2)
            ws_list[0] = load_qk_weights(0)
            x8sb = big.tile([P, KD, T], F8, name="x8sb", tag="x8sb", bufs=1)
            for kd in range(KD):
                nc.sync.dma_start(out=x8sb[:, kd, :],
                                  in_=x8_d[kd * P:(kd + 1) * P, :])
            ones = big.tile([P, P], BF16, name="ones_sb", tag="ones", bufs=1)
            nc.vector.memset(ones, 1.0)
            cmask = big.tile([P, P], BF16, name="cmask_sb", tag="cmask", bufs=1)
            nc.sync.dma_start(out=cmask, in_=cm_d)
            if H > 1:
                ws_list[1] = load_qk_weights(1)
            wv_sb = big.tile([P, KD, F], BF16, name="wv_sb", tag="wv", bufs=1)
            for kd in range(KD):
                nc.sync.dma_start(out=wv_sb[:, kd, :], in_=wvm_d[kd])
            vhb = big.tile([P, TT, F], BF16, name="vhb", tag="vhb", bufs=1)
            vh8 = big.tile([P, TT, F], F8, name="vh8", tag="vh8", bufs=1)
            otb = big.tile([P, H, 512], BF16, name="otb", tag="otb", bufs=1)
            ot8 = big.tile([P, H, T - 512], F8, name="ot8", tag="ot8", bufs=1)

            chunks = {}

            def load_chunk(tt):
                c = work.tile([P, KD * P], BF16, name=f"xtc{tt}", tag="xtc")
                nc.sync.dma_start(out=c, in_=xtc_d[tt])
                chunks[tt] = c

            load_chunk(0)

            # ---- fill generators (pumped between exp-gated attention ops)
            def qk_fill_gen(ws, tiles):
                wq_t, wk_t = ws
                qt_, kt_ = tiles
                for w_t, dst in ((wq_t, qt_), (wk_t, kt_)):
                    for tb in range(TB):
                        ts_ = slice(tb * 512, (tb + 1) * 512)
                        ps = psP.tile([P, 512], F32, name="psfill", tag="psp")
                        for j in range(JD):
                            nc.tensor.matmul(ps,
                                             lhsT=w_t[:, 2 * j:2 * j + 2, :],
                                             rhs=x8sb[:, 2 * j:2 * j + 2, ts_],
                                             start=(j == 0), stop=(j == JD - 1),
                                             perf_mode=DR)
                            yield
                        nc.vector.tensor_copy(out=dst[:, ts_], in_=ps)

            vdone = {}

            def v_gen(tt):
                """V projection for token-tile tt (all heads), token-major:
                x chunk stationary, w_v moving. Prefetches chunk tt+1."""
                if tt + 1 < TT:
                    load_chunk(tt + 1)
                c = chunks[tt]
                for fc in range(F // 512):
                    fs = slice(fc * 512, (fc + 1) * 512)
                    ps = psP.tile([P, 512], F32, name="psv", tag="psp")
                    for kd in range(KD):
                        nc.tensor.matmul(ps, lhsT=c[:, kd * P:(kd + 1) * P],
                                         rhs=wv_sb[:, kd, fs],
                                         start=(kd == 0), stop=(kd == KD - 1))
                        yield
                    nc.vector.tensor_copy(out=vhb[:, tt, fs], in_=ps)
                    nc.vector.tensor_copy(out=vh8[:, tt, fs], in_=ps)
                del chunks[tt]
                vdone[tt] = True

            def alloc_head_tiles(h):
                qt_ = work.tile([P, T], BF16, name=f"qt{h}", tag="qt")
                kt_ = work.tile([P, T], BF16, name=f"kt{h}", tag="kt")
                return qt_, kt_

            def oproj_nb_gen(nb):
                """o_proj chains for token block nb. nb=0 bf16, else fp8 DR."""
                ns = slice(nb * 512, (nb + 1) * 512)
                ns8 = slice((nb - 1) * 512, nb * 512)
                for m in range(MT):
                    if nb == 0:
                        wt = work.tile([P, H, P], BF16, name=f"wob{m}",
                                       tag="wob", bufs=3)
                        nc.sync.dma_start(
                            out=wt.rearrange("p h f -> p (h f)"), in_=wob_d[m])
                        yield  # cover the weight DMA latency
                        ps = psP.tile([P, 512], F32, name="psout", tag="psp")
                        for hh in range(H):
                            nc.tensor.matmul(ps, lhsT=wt[:, hh, :],
                                             rhs=otb[:, hh, :],
                                             start=(hh == 0),
                                             stop=(hh == H - 1))
                            yield
                    else:
                        wt = work.tile([P, H, P], F8, name=f"wo8_{nb}_{m}",
                                       tag="wo8", bufs=4)
                        nc.sync.dma_start(
                            out=wt.rearrange("p h f -> p (h f)"), in_=wo8_d[m])
                        yield
                        ps = psP.tile([P, 512], F32, name="psout", tag="psp")
                        for j in range(H // 2):
                            nc.tensor.matmul(ps, lhsT=wt[:, 2 * j:2 * j + 2, :],
                                             rhs=ot8[:, 2 * j:2 * j + 2, ns8],
                                             start=(j == 0),
                                             stop=(j == H // 2 - 1),
                                             perf_mode=DR)
                            yield
                    stg = work.tile([P, 512], F32, name="ostage", tag="ostage",
                                    bufs=3)
                    nc.vector.tensor_copy(out=stg, in_=ps)
                    nc.sync.dma_start(out=out_d[m * P:(m + 1) * P, ns],
                                      in_=stg)

            def pump(gen, n):
                for _ in range(n):
                    try:
                        next(gen)
                    except StopIteration:
                        return False
                return True

            def pump_n(gen, n):
                c = 0
                for _ in range(n):
                    try:
                        next(gen)
                        c += 1
                    except StopIteration:
                        break
                return c

            fills = []

            def pump_fills(n):
                while n > 0 and fills:
                    n -= pump_n(fills[0], n)
                    if n > 0:
                        fills.pop(0)

            def ensure_v(tt_hi):
                while not vdone.get(tt_hi, False) and fills:
                    pump_fills(16)

            # head-0 Q/K projections run to completion up front
            cur_tiles = alloc_head_tiles(0)
            g0 = qk_fill_gen(ws_list[0], cur_tiles)
            while pump(g0, 1):
                pass

            for h in range(H):
                qt_, kt_ = cur_tiles
                if h + 2 < H:
                    ws_list[h + 2] = load_qk_weights(h + 2)
                if h == 0:
                    for tt in range(TT):
                        fills.append(v_gen(tt))
                if h + 1 < H:
                    next_tiles = alloc_head_tiles(h + 1)
                    fills.append(qk_fill_gen(ws_list[h + 1], next_tiles))
                else:
                    next_tiles = None
                hs = slice(h * P, (h + 1) * P)

                for qb in range(TB):
                    if h == 0:
                        ensure_v(KT_PER_B * (qb + 1) - 1)
                    nkt = KT_PER_B * (qb + 1)
                    off = KT_PER_B * qb  # off-diagonal key tiles (fp8 DR)
                    qs_full = slice(qb * 512, (qb + 1) * 512)
                    psO_t = psO.tile([P, 512], F32, name="psodt", tag="pso")
                    racc = work.tile([P, 512], F32, name="racc", tag="racc")
                    for j in range(off // 2):
                        pt8_t = work.tile([P, 2, 512], F8, name="pt8",
                                          tag="pt8", bufs=3)
                        for u in range(2):
                            kt = 2 * j + u
                            psS_t = psS.tile([P, 512], F32, name="pssc",
                                             tag="pss")
                            nc.tensor.matmul(psS_t,
                                             lhsT=kt_[:, kt * P:(kt + 1) * P],
                                             rhs=qt_[:, qs_full],
                                             start=True, stop=True)
                            nc.scalar.activation(out=pt8_t[:, u, :],
                                                 in_=psS_t, func=EXP,
                                                 scale=scale)
                            if kt == 0:
                                nc.vector.tensor_copy(out=racc,
                                                      in_=pt8_t[:, u, :])
                            else:
                                nc.vector.tensor_add(racc, racc,
                                                     pt8_t[:, u, :])
                            pump_fills(1)
                        nc.tensor.matmul(psO_t,
                                         lhsT=vh8[:, 2 * j:2 * j + 2, hs],
                                         rhs=pt8_t,
                                         start=(j == 0), stop=False,
                                         perf_mode=DR, skip_group_check=True)
                        pump_fills(2)
                    # diagonal tiles: bf16, narrowed to live query columns
                    for kt in range(off, nkt):
                        di = kt - off
                        c0 = di * P
                        w = 512 - c0
                        qs = slice(qb * 512 + c0, (qb + 1) * 512)
                        psS_t = psS.tile([P, 512], F32, name="pssc", tag="pss")
                        nc.tensor.matmul(psS_t[:, :w],
                                         lhsT=kt_[:, kt * P:(kt + 1) * P],
                                         rhs=qt_[:, qs],
                                         start=True, stop=True)
                        ptb = work.tile([P, 512], BF16, name="pexp", tag="pt",
                                        bufs=4)
                        nc.scalar.activation(out=ptb[:, :w], in_=psS_t[:, :w],
                                             func=EXP, scale=scale)
                        nc.vector.tensor_mul(ptb[:, :P], ptb[:, :P], cmask)
                        nc.tensor.matmul(psO_t[:, c0:], lhsT=vhb[:, kt, hs],
                                         rhs=ptb[:, :w],
                                         start=(kt == 0),
                                         stop=(kt == nkt - 1),
                                         skip_group_check=True)
                        if kt == 0:
                            nc.vector.tensor_copy(out=racc, in_=ptb)
                        else:
                            nc.vector.tensor_add(racc[:, c0:], racc[:, c0:],
                                                 ptb[:, :w])
                        pump_fills(1 + (kt & 1))
                    raccb = work.tile([P, 512], BF16, name="raccb", tag="raccb")
                    nc.vector.tensor_copy(out=raccb, in_=racc)
                    psR_t = psR.tile([P, 512], F32, name="psrow", tag="psr")
                    nc.tensor.matmul(psR_t, lhsT=ones, rhs=raccb,
                                     start=True, stop=True)
                    rec = work.tile([P, 512], F32, name="rec", tag="rec")
                    nc.vector.reciprocal_approx_fast(out=rec, in_=psR_t)
                    if qb == 0:
                        nc.vector.tensor_mul(otb[:, h, :], psO_t, rec)
                    else:
                        nc.vector.tensor_mul(
                            ot8[:, h, (qb - 1) * 512:qb * 512], psO_t, rec)
                    if h == H - 1:
                        fills.append(oproj_nb_gen(qb))
                    pump_fills(4)
                if h < H - 1:
                    while fills:
                        pump_fills(64)
                cur_tiles = next_tiles
            while fills:
                pump_fills(64)

    nc.compile()
    return nc


def _ternary(w):
    """BitLinear ternary weights + gamma: clip(round(w/gamma),-1,1), gamma."""
    w = np.asarray(w, dtype=np.float32)
    gamma = max(np.float32(np.abs(w).mean(dtype=np.float32)), np.float32(1e-5))
    q = np.clip(np.round(w / gamma), -1.0, 1.0).astype(np.float32)
    return q, gamma


def _causal_mask():
    k = np.arange(128)[:, None]
    q = np.arange(128)[None, :]
    return (k <= q).astype(np.float32).astype(ml_dtypes.bfloat16)


def _tile_qkv(t_shard):
    """[F, D] -> [H, 128, KD*128]: [h, p, kd*128+f] = t[h*128+f, kd*128+p]."""
    Fs, Ds = t_shard.shape
    a = t_shard.reshape(Fs // 128, 128, Ds // 128, 128)  # [h, f, kd, p]
    a = a.transpose(0, 3, 2, 1).reshape(Fs // 128, 128, Ds)
    return np.ascontiguousarray(a)


def _tile_wo(t_shard):
    """[D, F] -> [MT, 128, H*128]: [m, p, h*128+j] = t[m*128+j, h*128+p]."""
    Ds, Fs = t_shard.shape
    a = t_shard.reshape(Ds // 128, 128, Fs // 128, 128)  # [m, j, h, p]
    a = a.transpose(0, 3, 2, 1).reshape(Ds // 128, 128, Fs)
    return np.ascontiguousarray(a)


def _tile_xtc(xb):
    """[T, D] -> [TT, 128, KD*128]: [tt, p, kd*128+j] = x[tt*128+j, kd*128+p]."""
    T, D = xb.shape
    a = xb.reshape(T // 128, 128, D // 128, 128)  # [tt, j, kd, p]
    a = a.transpose(0, 3, 2, 1).reshape(T // 128, 128, D)
    return np.ascontiguousarray(a)


def _prep_inputs(x, wq, wk, wv, wo):
    bf = ml_dtypes.bfloat16
    f8 = ml_dtypes.float8_e4m3
    x = np.asarray(x, dtype=np.float32)
    tq, gq = _ternary(wq)
    tk, gk = _ternary(wk)
    tv, gv = _ternary(wv)
    to, go = _ternary(wo)
    scale = float(gq) * float(gk) / math.sqrt(D_HEAD)
    oscale = 2.0 * float(gv) * float(go)
    cmask = _causal_mask()
    x8s = [np.ascontiguousarray(x[b].T).astype(f8) for b in range(B)]
    xtcs = [_tile_xtc(x[b]).astype(bf) for b in range(B)]
    shards = []
    for hg in range(2):
        rows = slice(hg * F_LOC, (hg + 1) * F_LOC)
        wvm = (0.5 * tv[rows, :]).T.reshape(D_MODEL // 128, 128, F_LOC)
        shards.append({
            "wq8": _tile_qkv(tq[rows, :]).astype(f8),
            "wk8": _tile_qkv(tk[rows, :]).astype(f8),
            "wvm": np.ascontiguousarray(wvm).astype(bf),
            "wo8": _tile_wo(to[:, rows]).astype(f8),
            "wob": _tile_wo(to[:, rows]).astype(bf),
        })
    in_maps = []
    for c in range(N_CORES):
        b, hg = c // 2, c % 2
        m = {"x8": x8s[b], "xtc": xtcs[b], "cmask": cmask}
        m.update(shards[hg])
        in_maps.append(m)
    return in_maps, scale, oscale


_NC_CACHE = {}


def _get_nc(scale):
    key = round(float(scale), 12)
    if key not in _NC_CACHE:
        _NC_CACHE[key] = build_bass(scale)
    return _NC_CACHE[key]


def run(x, wq, wk, wv, wo, trace=False):
    in_maps, scale, oscale = _prep_inputs(x, wq, wk, wv, wo)
    nc = _get_nc(scale)
    res = bass_utils.run_bass_kernel_spmd(
        nc, in_maps, core_ids=list(range(N_CORES)), trace=trace)
    out = np.empty((B, T_FULL, D_MODEL), dtype=np.float32)
    for b in range(B):
        out[b] = (res.results[2 * b]["outT"]
                  + res.results[2 * b + 1]["outT"]).T * oscale
    return out, res


def kernel(x, wq, wk, wv, wo):
    out, _ = run(x, wq, wk, wv, wo)
    return out
